# revision 3
# baseline (speedup 1.0000x reference)
"""Trainium2 Bass kernel for nn_DeepCausalModel (moe_routing).

Network (per row n of x [65536, 512]):
  trunk:  h0 = relu(x @ Wx0 + bx0)        512 -> 2048
          h1 = relu(h0 @ Wx1 + bx1)       2048 -> 2048
          emb = relu(h1 @ Wx2 + bx2)      2048 -> 512   (output x_emb)
  expert t=treatment[n]:
          g0 = relu(emb @ Wy0[t] + by0[t])  512 -> 1024
          g1 = relu(g0 @ Wy1[t] + by1[t])   1024 -> 512
          y  = g1 @ Wo[t] + bo[t]           512 -> 1    (output y)
  t_out = softmax(emb @ Wt + bt, axis=1) over a size-1 axis == 1.0 exactly.

Strategy: data-parallel over 8 NeuronCores (8192 rows each), all weights
SBUF-resident in fp16 (11-bit mantissa; fp32 PSUM accumulation), activations
kept feature-major [feat, rows] so no on-chip transposes (host pre-transposes
x, host re-transposes x_emb). Both experts are computed densely per row and
the result is selected on-chip with a fp32 treatment mask.
"""

import os
import sys
import time

for _p in ("/opt/trn_rl_repo", "/root/.axon_site/_ro/trn_rl_repo"):
    if os.path.isdir(_p) and _p not in sys.path:
        sys.path.append(_p)

import numpy as np

N_CORES = 8
N_TOTAL = 65536
D_IN = 512
D_H = 2048
D_EMB = 512
D_E0 = 1024
D_E1 = 512
T = 2

ROWS_PER_CORE = N_TOTAL // N_CORES  # 8192
NT = 512                            # rows per n-tile (matmul moving dim)


def build_module(ntiles=ROWS_PER_CORE // NT, repeat=1):
    """Build + finalize the per-core Bass module. ntiles*NT rows per core."""
    from contextlib import ExitStack

    import concourse.tile as tile
    from concourse import bacc, mybir
    from concourse.bass import ds

    FP16 = mybir.dt.float16
    F32 = mybir.dt.float32
    RELU = mybir.ActivationFunctionType.Relu

    rows = ntiles * NT

    nc = bacc.Bacc("TRN2", target_bir_lowering=False, debug=False,
                   num_devices=N_CORES)

    # ---- DRAM parameters (per-core shard) ----
    x_t = nc.declare_dram_parameter("x_t", [D_IN, rows], FP16, isOutput=False)
    w0 = nc.declare_dram_parameter("w0", [D_IN, D_H], FP16, isOutput=False)
    b0 = nc.declare_dram_parameter("b0", [D_H], F32, isOutput=False)
    w1 = nc.declare_dram_parameter("w1", [D_H, D_H], FP16, isOutput=False)
    b1 = nc.declare_dram_parameter("b1", [D_H], F32, isOutput=False)
    w2 = nc.declare_dram_parameter("w2", [D_H, D_EMB], FP16, isOutput=False)
    b2 = nc.declare_dram_parameter("b2", [D_EMB], F32, isOutput=False)
    wy0 = nc.declare_dram_parameter("wy0", [T, D_EMB, D_E0], FP16, isOutput=False)
    by0 = nc.declare_dram_parameter("by0", [T, D_E0], F32, isOutput=False)
    wy1 = nc.declare_dram_parameter("wy1", [T, D_E0, D_E1], FP16, isOutput=False)
    by1 = nc.declare_dram_parameter("by1", [T, D_E1], F32, isOutput=False)
    # Wo packed by host as [D_E1, T] (column t = head weights of treatment t)
    wo = nc.declare_dram_parameter("wo", [D_E1, T], FP16, isOutput=False)
    # Per-row fp32 treatment mask and per-row output bias bo[treatment[n]]
    tmask = nc.declare_dram_parameter("tmask", [rows], F32, isOutput=False)
    ybias = nc.declare_dram_parameter("ybias", [rows], F32, isOutput=False)

    emb_t = nc.declare_dram_parameter("emb_t", [D_EMB, rows], F32, isOutput=True)
    y = nc.declare_dram_parameter("y", [rows], F32, isOutput=True)

    KT0 = D_IN // 128    # 4   k-tiles into layer 0
    MT0 = D_H // 128     # 16  m-tiles out of layer 0
    KT1 = D_H // 128     # 16
    MT1 = D_H // 128     # 16
    MT2 = D_EMB // 128   # 4
    KTE = D_EMB // 128   # 4   k-tiles into expert l0
    MTE0 = D_E0 // 128   # 8
    KTE1 = D_E0 // 128   # 8
    MTE1 = D_E1 // 128   # 4
    KTH = D_E1 // 128    # 4   k-tiles into head

    with tile.TileContext(nc) as tc, ExitStack() as ctx:
        wpool = ctx.enter_context(tc.tile_pool(name="wpool", bufs=1))
        xpool = ctx.enter_context(tc.tile_pool(name="xpool", bufs=2))
        hpool = ctx.enter_context(tc.tile_pool(name="hpool", bufs=1))
        opool = ctx.enter_context(tc.tile_pool(name="opool", bufs=1))
        spool = ctx.enter_context(tc.tile_pool(name="spool", bufs=1))
        psum = ctx.enter_context(tc.tile_pool(name="psum", bufs=5, space="PSUM"))
        psum_h = ctx.enter_context(tc.tile_pool(name="psum_h", bufs=2, space="PSUM"))

        # ---- resident weights / biases ----
        w0_sb = wpool.tile([128, KT0, D_H], FP16)
        nc.sync.dma_start(out=w0_sb, in_=w0.rearrange("(kt p) m -> p kt m", p=128))
        w1_sb = wpool.tile([128, KT1, D_H], FP16)
        nc.sync.dma_start(out=w1_sb, in_=w1.rearrange("(kt p) m -> p kt m", p=128))
        w2_sb = wpool.tile([128, KT1, D_EMB], FP16)
        nc.sync.dma_start(out=w2_sb, in_=w2.rearrange("(kt p) m -> p kt m", p=128))
        wy0_sb = wpool.tile([128, T, KTE, D_E0], FP16)
        nc.sync.dma_start(out=wy0_sb,
                          in_=wy0.rearrange("t (kt p) m -> p t kt m", p=128))
        wy1_sb = wpool.tile([128, T, KTE1, D_E1], FP16)
        nc.sync.dma_start(out=wy1_sb,
                          in_=wy1.rearrange("t (kt p) m -> p t kt m", p=128))
        wo_sb = wpool.tile([128, KTH, T], FP16)
        nc.sync.dma_start(out=wo_sb, in_=wo.rearrange("(kt p) t -> p kt t", p=128))

        b0_sb = wpool.tile([128, MT0], F32)
        nc.sync.dma_start(out=b0_sb, in_=b0.rearrange("(mt p) -> p mt", p=128))
        b1_sb = wpool.tile([128, MT1], F32)
        nc.sync.dma_start(out=b1_sb, in_=b1.rearrange("(mt p) -> p mt", p=128))
        b2_sb = wpool.tile([128, MT2], F32)
        nc.sync.dma_start(out=b2_sb, in_=b2.rearrange("(mt p) -> p mt", p=128))
        by0_sb = wpool.tile([128, T, MTE0], F32)
        nc.sync.dma_start(out=by0_sb,
                          in_=by0.rearrange("t (mt p) -> p t mt", p=128))
        by1_sb = wpool.tile([128, T, MTE1], F32)
        nc.sync.dma_start(out=by1_sb,
                          in_=by1.rearrange("t (mt p) -> p t mt", p=128))

        x_t_r = x_t.rearrange("(kt p) n -> p kt n", p=128)
        emb_t_r = emb_t.rearrange("(mt p) n -> p mt n", p=128)

        def body(i):
            # ---- load x tile (feature-major) ----
            x_sb = xpool.tile([128, KT0, NT], FP16)
            nc.sync.dma_start(out=x_sb, in_=x_t_r[:, :, ds(i, NT)])
            mask_sb = spool.tile([1, NT], F32)
            nc.sync.dma_start(out=mask_sb, in_=tmask[ds(i, NT)])
            ybias_sb = spool.tile([1, NT], F32)
            nc.sync.dma_start(out=ybias_sb, in_=ybias[ds(i, NT)])

            # ---- trunk layer 0: 512 -> 2048 ----
            h0_sb = hpool.tile([128, MT0, NT], FP16, tag="h0")
            for m in range(MT0):
                acc = psum.tile([128, NT], F32, tag="acc")
                for k in range(KT0):
                    nc.tensor.matmul(acc, lhsT=w0_sb[:, k, m * 128:(m + 1) * 128],
                                     rhs=x_sb[:, k, :],
                                     start=(k == 0), stop=(k == KT0 - 1))
                nc.scalar.activation(out=h0_sb[:, m, :], in_=acc, func=RELU,
                                     bias=b0_sb[:, m:m + 1])

            # ---- trunk layer 1: 2048 -> 2048 ----
            h1_sb = hpool.tile([128, MT1, NT], FP16, tag="h1")
            for m in range(MT1):
                acc = psum.tile([128, NT], F32, tag="acc")
                for k in range(KT1):
                    nc.tensor.matmul(acc, lhsT=w1_sb[:, k, m * 128:(m + 1) * 128],
                                     rhs=h0_sb[:, k, :],
                                     start=(k == 0), stop=(k == KT1 - 1))
                nc.scalar.activation(out=h1_sb[:, m, :], in_=acc, func=RELU,
                                     bias=b1_sb[:, m:m + 1])

            # ---- trunk layer 2: 2048 -> 512 (x_emb) ----
            emb16_sb = hpool.tile([128, MT2, NT], FP16, tag="emb16")
            emb32_sb = opool.tile([128, MT2, NT], F32, tag="emb32")
            for m in range(MT2):
                acc = psum.tile([128, NT], F32, tag="acc")
                for k in range(KT1):
                    nc.tensor.matmul(acc, lhsT=w2_sb[:, k, m * 128:(m + 1) * 128],
                                     rhs=h1_sb[:, k, :],
                                     start=(k == 0), stop=(k == KT1 - 1))
                nc.scalar.activation(out=emb16_sb[:, m, :], in_=acc, func=RELU,
                                     bias=b2_sb[:, m:m + 1])
                nc.scalar.activation(out=emb32_sb[:, m, :], in_=acc, func=RELU,
                                     bias=b2_sb[:, m:m + 1])
            nc.sync.dma_start(out=emb_t_r[:, :, ds(i, NT)], in_=emb32_sb)

            # ---- experts (dense in T) + head ----
            y_ps = []
            for t in range(T):
                e0_sb = hpool.tile([128, MTE0, NT], FP16, tag="e0")
                for m in range(MTE0):
                    acc = psum.tile([128, NT], F32, tag="acc")
                    for k in range(KTE):
                        nc.tensor.matmul(
                            acc, lhsT=wy0_sb[:, t, k, m * 128:(m + 1) * 128],
                            rhs=emb16_sb[:, k, :],
                            start=(k == 0), stop=(k == KTE - 1))
                    nc.scalar.activation(out=e0_sb[:, m, :], in_=acc, func=RELU,
                                         bias=by0_sb[:, t, m:m + 1])
                e1_sb = hpool.tile([128, MTE1, NT], FP16, tag="e1")
                for m in range(MTE1):
                    acc = psum.tile([128, NT], F32, tag="acc")
                    for k in range(KTE1):
                        nc.tensor.matmul(
                            acc, lhsT=wy1_sb[:, t, k, m * 128:(m + 1) * 128],
                            rhs=e0_sb[:, k, :],
                            start=(k == 0), stop=(k == KTE1 - 1))
                    nc.scalar.activation(out=e1_sb[:, m, :], in_=acc, func=RELU,
                                         bias=by1_sb[:, t, m:m + 1])
                y_acc = psum_h.tile([1, NT], F32, tag="y_acc")
                for k in range(KTH):
                    nc.tensor.matmul(y_acc, lhsT=wo_sb[:, k, t:t + 1],
                                     rhs=e1_sb[:, k, :],
                                     start=(k == 0), stop=(k == KTH - 1))
                y_ps.append(y_acc)

            # ---- select by treatment: y = y0 + mask*(y1-y0) + bo[treatment] ----
            # (only one DVE input may be PSUM -> copy y0 to SBUF first)
            y0_sb = spool.tile([1, NT], F32)
            nc.vector.tensor_copy(y0_sb, y_ps[0])
            d_sb = spool.tile([1, NT], F32)
            nc.vector.tensor_sub(d_sb, y_ps[1], y0_sb)
            nc.vector.tensor_mul(d_sb, d_sb, mask_sb)
            y_sb = spool.tile([1, NT], F32)
            nc.vector.tensor_add(y_sb, y0_sb, d_sb)
            nc.vector.tensor_add(y_sb, y_sb, ybias_sb)
            nc.sync.dma_start(out=y[ds(i, NT)], in_=y_sb)

        def n_loop():
            with tc.For_i(0, rows, NT,
                          hint_engines=(mybir.EngineType.PE,)) as i:
                body(i)

        if repeat == 1:
            n_loop()
        else:
            with tc.For_i(0, repeat, 1):
                n_loop()

    nc.finalize()
    return nc


# ---------------------------------------------------------------------------
# Host side
# ---------------------------------------------------------------------------

_RUNNER_CACHE = {}


def _make_runner(nc):
    """Reusable jitted runner for a finalized Bass module (8-core SPMD)."""
    import jax
    from jax.sharding import Mesh, PartitionSpec
    from jax.experimental.shard_map import shard_map

    from concourse import bass2jax, mybir
    from concourse.bass2jax import _bass_exec_p, install_neuronx_cc_hook

    install_neuronx_cc_hook()

    partition_name = nc.partition_id_tensor.name if nc.partition_id_tensor else None
    in_names, out_names, out_avals = [], [], []
    for alloc in nc.m.functions[0].allocations:
        if not isinstance(alloc, mybir.MemoryLocationSet):
            continue
        name = alloc.memorylocations[0].name
        if alloc.kind == "ExternalInput":
            if name != partition_name:
                in_names.append(name)
        elif alloc.kind == "ExternalOutput":
            out_names.append(name)
            out_avals.append(
                jax.core.ShapedArray(tuple(alloc.tensor_shape),
                                     mybir.dt.np(alloc.dtype)))
    n_params = len(in_names)
    all_in_names = in_names + out_names
    if partition_name is not None:
        all_in_names = all_in_names + [partition_name]

    def _body(*args):
        operands = list(args)
        if partition_name is not None:
            operands.append(bass2jax.partition_id_tensor())
        return tuple(_bass_exec_p.bind(
            *operands,
            out_avals=tuple(out_avals),
            in_names=tuple(all_in_names),
            out_names=tuple(out_names),
            lowering_input_output_aliases=(),
            sim_require_finite=True,
            sim_require_nnan=True,
            nc=nc,
        ))

    devices = jax.devices()[:N_CORES]
    mesh = Mesh(np.asarray(devices), ("core",))
    n_outs = len(out_names)
    fn = jax.jit(
        shard_map(_body, mesh=mesh,
                  in_specs=(PartitionSpec("core"),) * (n_params + n_outs),
                  out_specs=(PartitionSpec("core"),) * n_outs,
                  check_rep=False),
        keep_unused=True,
    )

    def run(in_maps, n_timed=0):
        import jax
        concat_in = [
            np.concatenate([np.asarray(in_maps[c][nm]) for c in range(N_CORES)],
                           axis=0)
            for nm in in_names
        ]
        zeros = [np.zeros((N_CORES * a.shape[0],) + tuple(a.shape[1:]), a.dtype)
                 for a in out_avals]
        args = [jax.device_put(a) for a in concat_in + zeros]
        jax.block_until_ready(args)
        outs = jax.block_until_ready(fn(*args))
        times = []
        for _ in range(n_timed):
            t0 = time.perf_counter()
            jax.block_until_ready(fn(*args))
            times.append(time.perf_counter() - t0)
        outs_np = [np.asarray(o) for o in outs]
        res = []
        for c in range(N_CORES):
            d = {}
            for nm, o, av in zip(out_names, outs_np, out_avals):
                per = av.shape[0]
                d[nm] = o[c * per:(c + 1) * per]
            res.append(d)
        return res, times

    return run


def _get_runner(ntiles=ROWS_PER_CORE // NT, repeat=1):
    key = (ntiles, repeat)
    if key not in _RUNNER_CACHE:
        import jax
        try:
            jax.config.update("jax_compilation_cache_dir",
                              os.path.expanduser("~/.cache/bass_jax_cache"))
            jax.config.update("jax_persistent_cache_min_entry_size_bytes", -1)
            jax.config.update("jax_persistent_cache_min_compile_time_secs", 0)
        except Exception:
            pass
        nc = build_module(ntiles=ntiles, repeat=repeat)
        _RUNNER_CACHE[key] = _make_runner(nc)
    return _RUNNER_CACHE[key]


def prepare_in_maps(x, treatment, Wx0, bx0, Wx1, bx1, Wx2, bx2,
                    Wy0, by0, Wy1, by1, Wo, bo):
    """Shard + pre-transpose inputs into per-core input maps."""
    f16 = np.float16
    x = np.asarray(x, dtype=np.float32)
    tr = np.asarray(treatment).astype(np.int64)
    shared = {
        "w0": np.asarray(Wx0, np.float32).astype(f16),
        "b0": np.ascontiguousarray(np.asarray(bx0, np.float32)),
        "w1": np.asarray(Wx1, np.float32).astype(f16),
        "b1": np.ascontiguousarray(np.asarray(bx1, np.float32)),
        "w2": np.asarray(Wx2, np.float32).astype(f16),
        "b2": np.ascontiguousarray(np.asarray(bx2, np.float32)),
        "wy0": np.asarray(Wy0, np.float32).astype(f16),
        "by0": np.ascontiguousarray(np.asarray(by0, np.float32)),
        "wy1": np.asarray(Wy1, np.float32).astype(f16),
        "by1": np.ascontiguousarray(np.asarray(by1, np.float32)),
        "wo": np.ascontiguousarray(np.asarray(Wo, np.float32)[:, :, 0].T).astype(f16),
    }
    bo_flat = np.asarray(bo, np.float32).reshape(T)
    in_maps = []
    for c in range(N_CORES):
        sl = slice(c * ROWS_PER_CORE, (c + 1) * ROWS_PER_CORE)
        m = dict(shared)
        m["x_t"] = np.ascontiguousarray(x[sl].T).astype(f16)
        m["tmask"] = tr[sl].astype(np.float32)
        m["ybias"] = bo_flat[tr[sl]]
        in_maps.append(m)
    return in_maps


def kernel(x, treatment, Wx0, bx0, Wx1, bx1, Wx2, bx2,
           Wy0, by0, Wy1, by1, Wo, bo, Wt, bt):
    in_maps = prepare_in_maps(x, treatment, Wx0, bx0, Wx1, bx1, Wx2, bx2,
                              Wy0, by0, Wy1, by1, Wo, bo)
    run = _get_runner()
    res, _ = run(in_maps)

    n = np.asarray(x).shape[0]
    y_full = np.empty((n, 1), np.float32)
    emb_full = np.empty((n, D_EMB), np.float32)
    for c in range(N_CORES):
        sl = slice(c * ROWS_PER_CORE, (c + 1) * ROWS_PER_CORE)
        y_full[sl, 0] = res[c]["y"]
        emb_full[sl] = res[c]["emb_t"].T
    t_out = np.ones((n, 1), np.float32)
    return y_full, emb_full, t_out


# revision 4
# speedup vs baseline: 114.4708x; 114.4708x over previous
"""Trainium2 Bass kernel for nn_DeepCausalModel (moe_routing).

Network (per row n of x [65536, 512]):
  trunk:  h0 = relu(x @ Wx0 + bx0)        512 -> 2048
          h1 = relu(h0 @ Wx1 + bx1)       2048 -> 2048
          emb = relu(h1 @ Wx2 + bx2)      2048 -> 512   (output x_emb)
  expert t=treatment[n]:
          g0 = relu(emb @ Wy0[t] + by0[t])  512 -> 1024
          g1 = relu(g0 @ Wy1[t] + by1[t])   1024 -> 512
          y  = g1 @ Wo[t] + bo[t]           512 -> 1    (output y)
  t_out = softmax(emb @ Wt + bt, axis=1) over a size-1 axis == 1.0 exactly.

Strategy: data-parallel over 8 NeuronCores (8192 rows each), all weights
SBUF-resident in fp16 (11-bit mantissa; fp32 PSUM accumulation), activations
kept feature-major [feat, rows] so no on-chip transposes (host pre-transposes
x, host re-transposes x_emb). Both experts are computed densely per row and
the result is selected on-chip with a fp32 treatment mask.
"""

import os
import sys
import time

for _p in ("/opt/trn_rl_repo", "/root/.axon_site/_ro/trn_rl_repo"):
    if os.path.isdir(_p) and _p not in sys.path:
        sys.path.append(_p)

import numpy as np

N_CORES = 8
N_TOTAL = 65536
D_IN = 512
D_H = 2048
D_EMB = 512
D_E0 = 1024
D_E1 = 512
T = 2

ROWS_PER_CORE = N_TOTAL // N_CORES  # 8192
NT = 512                            # rows per n-tile (matmul moving dim)


def build_module(ntiles=ROWS_PER_CORE // NT, repeat=1):
    """Build + finalize the per-core Bass module. ntiles*NT rows per core."""
    from contextlib import ExitStack

    import concourse.tile as tile
    from concourse import bacc, mybir
    from concourse.bass import ds

    FP16 = mybir.dt.float16
    F32 = mybir.dt.float32
    RELU = mybir.ActivationFunctionType.Relu

    rows = ntiles * NT

    nc = bacc.Bacc("TRN2", target_bir_lowering=False, debug=False,
                   num_devices=N_CORES)

    # ---- DRAM parameters (per-core shard) ----
    x_t = nc.declare_dram_parameter("x_t", [D_IN, rows], FP16, isOutput=False)
    w0 = nc.declare_dram_parameter("w0", [D_IN, D_H], FP16, isOutput=False)
    b0 = nc.declare_dram_parameter("b0", [D_H], F32, isOutput=False)
    w1 = nc.declare_dram_parameter("w1", [D_H, D_H], FP16, isOutput=False)
    b1 = nc.declare_dram_parameter("b1", [D_H], F32, isOutput=False)
    w2 = nc.declare_dram_parameter("w2", [D_H, D_EMB], FP16, isOutput=False)
    b2 = nc.declare_dram_parameter("b2", [D_EMB], F32, isOutput=False)
    wy0 = nc.declare_dram_parameter("wy0", [T, D_EMB, D_E0], FP16, isOutput=False)
    by0 = nc.declare_dram_parameter("by0", [T, D_E0], F32, isOutput=False)
    wy1 = nc.declare_dram_parameter("wy1", [T, D_E0, D_E1], FP16, isOutput=False)
    by1 = nc.declare_dram_parameter("by1", [T, D_E1], F32, isOutput=False)
    # Wo packed by host as [D_E1, T] (column t = head weights of treatment t)
    wo = nc.declare_dram_parameter("wo", [D_E1, T], FP16, isOutput=False)
    # Per-row fp32 treatment mask and per-row output bias bo[treatment[n]]
    tmask = nc.declare_dram_parameter("tmask", [rows], F32, isOutput=False)
    ybias = nc.declare_dram_parameter("ybias", [rows], F32, isOutput=False)

    emb_t = nc.declare_dram_parameter("emb_t", [D_EMB, rows], F32, isOutput=True)
    y = nc.declare_dram_parameter("y", [rows], F32, isOutput=True)

    KT0 = D_IN // 128    # 4   k-tiles into layer 0
    MT0 = D_H // 128     # 16  m-tiles out of layer 0
    KT1 = D_H // 128     # 16
    MT1 = D_H // 128     # 16
    MT2 = D_EMB // 128   # 4
    KTE = D_EMB // 128   # 4   k-tiles into expert l0
    MTE0 = D_E0 // 128   # 8
    KTE1 = D_E0 // 128   # 8
    MTE1 = D_E1 // 128   # 4
    KTH = D_E1 // 128    # 4   k-tiles into head

    with tile.TileContext(nc) as tc, ExitStack() as ctx:
        wpool = ctx.enter_context(tc.tile_pool(name="wpool", bufs=1))
        xpool = ctx.enter_context(tc.tile_pool(name="xpool", bufs=2))
        hpool = ctx.enter_context(tc.tile_pool(name="hpool", bufs=1))
        opool = ctx.enter_context(tc.tile_pool(name="opool", bufs=1))
        spool = ctx.enter_context(tc.tile_pool(name="spool", bufs=1))
        psum = ctx.enter_context(tc.tile_pool(name="psum", bufs=5, space="PSUM"))
        psum_h = ctx.enter_context(tc.tile_pool(name="psum_h", bufs=2, space="PSUM"))

        # ---- resident weights / biases ----
        w0_sb = wpool.tile([128, KT0, D_H], FP16)
        nc.sync.dma_start(out=w0_sb, in_=w0.rearrange("(kt p) m -> p kt m", p=128))
        w1_sb = wpool.tile([128, KT1, D_H], FP16)
        nc.sync.dma_start(out=w1_sb, in_=w1.rearrange("(kt p) m -> p kt m", p=128))
        w2_sb = wpool.tile([128, KT1, D_EMB], FP16)
        nc.sync.dma_start(out=w2_sb, in_=w2.rearrange("(kt p) m -> p kt m", p=128))
        wy0_sb = wpool.tile([128, T, KTE, D_E0], FP16)
        nc.sync.dma_start(out=wy0_sb,
                          in_=wy0.rearrange("t (kt p) m -> p t kt m", p=128))
        wy1_sb = wpool.tile([128, T, KTE1, D_E1], FP16)
        nc.sync.dma_start(out=wy1_sb,
                          in_=wy1.rearrange("t (kt p) m -> p t kt m", p=128))
        wo_sb = wpool.tile([128, KTH, T], FP16)
        nc.sync.dma_start(out=wo_sb, in_=wo.rearrange("(kt p) t -> p kt t", p=128))

        b0_sb = wpool.tile([128, MT0], F32)
        nc.sync.dma_start(out=b0_sb, in_=b0.rearrange("(mt p) -> p mt", p=128))
        b1_sb = wpool.tile([128, MT1], F32)
        nc.sync.dma_start(out=b1_sb, in_=b1.rearrange("(mt p) -> p mt", p=128))
        b2_sb = wpool.tile([128, MT2], F32)
        nc.sync.dma_start(out=b2_sb, in_=b2.rearrange("(mt p) -> p mt", p=128))
        by0_sb = wpool.tile([128, T, MTE0], F32)
        nc.sync.dma_start(out=by0_sb,
                          in_=by0.rearrange("t (mt p) -> p t mt", p=128))
        by1_sb = wpool.tile([128, T, MTE1], F32)
        nc.sync.dma_start(out=by1_sb,
                          in_=by1.rearrange("t (mt p) -> p t mt", p=128))

        x_t_r = x_t.rearrange("(kt p) n -> p kt n", p=128)
        emb_t_r = emb_t.rearrange("(mt p) n -> p mt n", p=128)

        def body(i):
            # ---- load x tile (feature-major) ----
            x_sb = xpool.tile([128, KT0, NT], FP16)
            nc.sync.dma_start(out=x_sb, in_=x_t_r[:, :, ds(i, NT)])
            mask_sb = spool.tile([1, NT], F32)
            nc.sync.dma_start(out=mask_sb, in_=tmask[ds(i, NT)])
            ybias_sb = spool.tile([1, NT], F32)
            nc.sync.dma_start(out=ybias_sb, in_=ybias[ds(i, NT)])

            # ---- trunk layer 0: 512 -> 2048 ----
            h0_sb = hpool.tile([128, MT0, NT], FP16, tag="h0")
            for m in range(MT0):
                acc = psum.tile([128, NT], F32, tag="acc")
                for k in range(KT0):
                    nc.tensor.matmul(acc, lhsT=w0_sb[:, k, m * 128:(m + 1) * 128],
                                     rhs=x_sb[:, k, :],
                                     start=(k == 0), stop=(k == KT0 - 1))
                nc.scalar.activation(out=h0_sb[:, m, :], in_=acc, func=RELU,
                                     bias=b0_sb[:, m:m + 1])

            # ---- trunk layer 1: 2048 -> 2048 ----
            h1_sb = hpool.tile([128, MT1, NT], FP16, tag="h1")
            for m in range(MT1):
                acc = psum.tile([128, NT], F32, tag="acc")
                for k in range(KT1):
                    nc.tensor.matmul(acc, lhsT=w1_sb[:, k, m * 128:(m + 1) * 128],
                                     rhs=h0_sb[:, k, :],
                                     start=(k == 0), stop=(k == KT1 - 1))
                nc.scalar.activation(out=h1_sb[:, m, :], in_=acc, func=RELU,
                                     bias=b1_sb[:, m:m + 1])

            # ---- trunk layer 2: 2048 -> 512 (x_emb) ----
            emb16_sb = hpool.tile([128, MT2, NT], FP16, tag="emb16")
            emb32_sb = opool.tile([128, MT2, NT], F32, tag="emb32")
            for m in range(MT2):
                acc = psum.tile([128, NT], F32, tag="acc")
                for k in range(KT1):
                    nc.tensor.matmul(acc, lhsT=w2_sb[:, k, m * 128:(m + 1) * 128],
                                     rhs=h1_sb[:, k, :],
                                     start=(k == 0), stop=(k == KT1 - 1))
                nc.scalar.activation(out=emb16_sb[:, m, :], in_=acc, func=RELU,
                                     bias=b2_sb[:, m:m + 1])
                nc.scalar.activation(out=emb32_sb[:, m, :], in_=acc, func=RELU,
                                     bias=b2_sb[:, m:m + 1])
            nc.sync.dma_start(out=emb_t_r[:, :, ds(i, NT)], in_=emb32_sb)

            # ---- experts (dense in T) + head ----
            y_ps = []
            for t in range(T):
                e0_sb = hpool.tile([128, MTE0, NT], FP16, tag="e0")
                for m in range(MTE0):
                    acc = psum.tile([128, NT], F32, tag="acc")
                    for k in range(KTE):
                        nc.tensor.matmul(
                            acc, lhsT=wy0_sb[:, t, k, m * 128:(m + 1) * 128],
                            rhs=emb16_sb[:, k, :],
                            start=(k == 0), stop=(k == KTE - 1))
                    nc.scalar.activation(out=e0_sb[:, m, :], in_=acc, func=RELU,
                                         bias=by0_sb[:, t, m:m + 1])
                e1_sb = hpool.tile([128, MTE1, NT], FP16, tag="e1")
                for m in range(MTE1):
                    acc = psum.tile([128, NT], F32, tag="acc")
                    for k in range(KTE1):
                        nc.tensor.matmul(
                            acc, lhsT=wy1_sb[:, t, k, m * 128:(m + 1) * 128],
                            rhs=e0_sb[:, k, :],
                            start=(k == 0), stop=(k == KTE1 - 1))
                    nc.scalar.activation(out=e1_sb[:, m, :], in_=acc, func=RELU,
                                         bias=by1_sb[:, t, m:m + 1])
                y_acc = psum_h.tile([1, NT], F32, tag="y_acc")
                for k in range(KTH):
                    nc.tensor.matmul(y_acc, lhsT=wo_sb[:, k, t:t + 1],
                                     rhs=e1_sb[:, k, :],
                                     start=(k == 0), stop=(k == KTH - 1))
                y_ps.append(y_acc)

            # ---- select by treatment: y = y0 + mask*(y1-y0) + bo[treatment] ----
            # (only one DVE input may be PSUM -> copy y0 to SBUF first)
            y0_sb = spool.tile([1, NT], F32)
            nc.vector.tensor_copy(y0_sb, y_ps[0])
            d_sb = spool.tile([1, NT], F32)
            nc.vector.tensor_sub(d_sb, y_ps[1], y0_sb)
            nc.vector.tensor_mul(d_sb, d_sb, mask_sb)
            y_sb = spool.tile([1, NT], F32)
            nc.vector.tensor_add(y_sb, y0_sb, d_sb)
            nc.vector.tensor_add(y_sb, y_sb, ybias_sb)
            nc.sync.dma_start(out=y[ds(i, NT)], in_=y_sb)

        def n_loop():
            with tc.For_i(0, rows, NT,
                          hint_engines=(mybir.EngineType.PE,)) as i:
                body(i)

        if repeat == 1:
            n_loop()
        else:
            with tc.For_i(0, repeat, 1):
                n_loop()

    nc.finalize()
    return nc


# ---------------------------------------------------------------------------
# Host side
# ---------------------------------------------------------------------------

_RUNNER_CACHE = {}


def _make_runner(nc):
    """Reusable jitted runner for a finalized Bass module (8-core SPMD)."""
    import jax
    from jax.sharding import Mesh, PartitionSpec
    from jax.experimental.shard_map import shard_map

    from concourse import bass2jax, mybir
    from concourse.bass2jax import _bass_exec_p, install_neuronx_cc_hook

    install_neuronx_cc_hook()

    partition_name = nc.partition_id_tensor.name if nc.partition_id_tensor else None
    in_names, out_names, out_avals = [], [], []
    for alloc in nc.m.functions[0].allocations:
        if not isinstance(alloc, mybir.MemoryLocationSet):
            continue
        name = alloc.memorylocations[0].name
        if alloc.kind == "ExternalInput":
            if name != partition_name:
                in_names.append(name)
        elif alloc.kind == "ExternalOutput":
            out_names.append(name)
            out_avals.append(
                jax.core.ShapedArray(tuple(alloc.tensor_shape),
                                     mybir.dt.np(alloc.dtype)))
    n_params = len(in_names)
    all_in_names = in_names + out_names
    if partition_name is not None:
        all_in_names = all_in_names + [partition_name]

    def _body(*args):
        operands = list(args)
        if partition_name is not None:
            operands.append(bass2jax.partition_id_tensor())
        return tuple(_bass_exec_p.bind(
            *operands,
            out_avals=tuple(out_avals),
            in_names=tuple(all_in_names),
            out_names=tuple(out_names),
            lowering_input_output_aliases=(),
            sim_require_finite=True,
            sim_require_nnan=True,
            nc=nc,
        ))

    devices = jax.devices()[:N_CORES]
    mesh = Mesh(np.asarray(devices), ("core",))
    n_outs = len(out_names)
    fn = jax.jit(
        shard_map(_body, mesh=mesh,
                  in_specs=(PartitionSpec("core"),) * (n_params + n_outs),
                  out_specs=(PartitionSpec("core"),) * n_outs,
                  check_rep=False),
        keep_unused=True,
    )

    def run(in_maps, n_timed=0):
        import jax
        from jax.sharding import NamedSharding, PartitionSpec as P
        concat_in = [
            np.concatenate([np.asarray(in_maps[c][nm]) for c in range(N_CORES)],
                           axis=0)
            for nm in in_names
        ]
        zeros = [np.zeros((N_CORES * a.shape[0],) + tuple(a.shape[1:]), a.dtype)
                 for a in out_avals]
        sh = NamedSharding(mesh, P("core"))
        args = [jax.device_put(a, sh) for a in concat_in + zeros]
        jax.block_until_ready(args)
        outs = jax.block_until_ready(fn(*args))
        times = []
        for _ in range(n_timed):
            t0 = time.perf_counter()
            jax.block_until_ready(fn(*args))
            times.append(time.perf_counter() - t0)
        outs_np = [np.asarray(o) for o in outs]
        res = []
        for c in range(N_CORES):
            d = {}
            for nm, o, av in zip(out_names, outs_np, out_avals):
                per = av.shape[0]
                d[nm] = o[c * per:(c + 1) * per]
            res.append(d)
        return res, times

    return run


def _get_runner(ntiles=ROWS_PER_CORE // NT, repeat=1):
    key = (ntiles, repeat)
    if key not in _RUNNER_CACHE:
        import jax
        try:
            jax.config.update("jax_compilation_cache_dir",
                              os.path.expanduser("~/.cache/bass_jax_cache"))
            jax.config.update("jax_persistent_cache_min_entry_size_bytes", -1)
            jax.config.update("jax_persistent_cache_min_compile_time_secs", 0)
        except Exception:
            pass
        nc = build_module(ntiles=ntiles, repeat=repeat)
        _RUNNER_CACHE[key] = _make_runner(nc)
    return _RUNNER_CACHE[key]


def prepare_in_maps(x, treatment, Wx0, bx0, Wx1, bx1, Wx2, bx2,
                    Wy0, by0, Wy1, by1, Wo, bo):
    """Shard + pre-transpose inputs into per-core input maps."""
    f16 = np.float16
    x = np.asarray(x, dtype=np.float32)
    tr = np.asarray(treatment).astype(np.int64)
    shared = {
        "w0": np.asarray(Wx0, np.float32).astype(f16),
        "b0": np.ascontiguousarray(np.asarray(bx0, np.float32)),
        "w1": np.asarray(Wx1, np.float32).astype(f16),
        "b1": np.ascontiguousarray(np.asarray(bx1, np.float32)),
        "w2": np.asarray(Wx2, np.float32).astype(f16),
        "b2": np.ascontiguousarray(np.asarray(bx2, np.float32)),
        "wy0": np.asarray(Wy0, np.float32).astype(f16),
        "by0": np.ascontiguousarray(np.asarray(by0, np.float32)),
        "wy1": np.asarray(Wy1, np.float32).astype(f16),
        "by1": np.ascontiguousarray(np.asarray(by1, np.float32)),
        "wo": np.ascontiguousarray(np.asarray(Wo, np.float32)[:, :, 0].T).astype(f16),
    }
    bo_flat = np.asarray(bo, np.float32).reshape(T)
    in_maps = []
    for c in range(N_CORES):
        sl = slice(c * ROWS_PER_CORE, (c + 1) * ROWS_PER_CORE)
        m = dict(shared)
        m["x_t"] = np.ascontiguousarray(x[sl].T).astype(f16)
        m["tmask"] = tr[sl].astype(np.float32)
        m["ybias"] = bo_flat[tr[sl]]
        in_maps.append(m)
    return in_maps


def kernel(x, treatment, Wx0, bx0, Wx1, bx1, Wx2, bx2,
           Wy0, by0, Wy1, by1, Wo, bo, Wt, bt):
    in_maps = prepare_in_maps(x, treatment, Wx0, bx0, Wx1, bx1, Wx2, bx2,
                              Wy0, by0, Wy1, by1, Wo, bo)
    run = _get_runner()
    res, _ = run(in_maps)

    n = np.asarray(x).shape[0]
    y_full = np.empty((n, 1), np.float32)
    emb_full = np.empty((n, D_EMB), np.float32)
    for c in range(N_CORES):
        sl = slice(c * ROWS_PER_CORE, (c + 1) * ROWS_PER_CORE)
        y_full[sl, 0] = res[c]["y"]
        emb_full[sl] = res[c]["emb_t"].T
    t_out = np.ones((n, 1), np.float32)
    return y_full, emb_full, t_out
